# revision 28
# baseline (speedup 1.0000x reference)
"""Trainium2 Bass kernel for the KAN layer (nn_KANLayer).

Math restructure
----------------
Reference computes, for x in [0,1) on a uniform extended B-spline grid
(g0 = grid[0,0], h = grid spacing, t = (x-g0)/h in [7,11), t' = t-9):

  y[b,o] = sum_i mask[i,o]*(scale_base[i,o]*silu(x[b,i])
                            + scale_sp[i,o]*sum_k basis_k(x[b,i])*coef[i,o,k])

On the restricted domain every cubic B-spline basis function is an exact
linear combination of the monomials {1, t', t'^2, t'^3} plus the three
relu-kink cubics relu(t'+1)^3, relu(t')^3, relu(t'-1)^3.  silu(x) on
[0,1) is replaced by its least-squares cubic (max err ~2e-4, well under
the 2e-2 gate) and folded into the monomial planes.  Since cubing
preserves sign, relu(t')^3 = relu(t'^3) costs one max after t'^3, and
relu(u)^3 = u^2*relu(u) gives the shifted kinks from an ACT Square and a
relu.  Net: SIX feature planes

  phi = [t', t'^2, t'^3, relu(t'+1)^3, relu(t')^3, relu(t'-1)^3]

and the whole layer is one accumulated matmul  y = Phi(x) @ W_fold + bias
with all scale factors folded into W on the host.

Sharding: out_dim split x4, batch split x2 -> 8 cores, no collectives.
Per core: x [128,1024] fp16 (2 chunked DMAs, SP ring) + W [128,3072] fp16
(2 DMAs, ACT ring).  Features are computed in 2 free-dim chunks balanced
across DVE (t', power chain, abs shifts), ACT (biased Squares) and Pool
(the two independent tensor_tensor products), so the 24 accumulated
matmuls (PE, fp16) start while the second chunk is still streaming.
Output is fp16 (cast to fp32 on host), bias added on ACT from PSUM.

Host does only weight folding, slicing, dtype casts and layout swizzles;
all per-token math (features, matmul, bias) runs on device.
"""

import sys

for _p in ("/opt/trn_rl_repo", "/opt/trn_rl_repo/concourse"):
    if _p not in sys.path:
        sys.path.insert(0, _p)

import numpy as np

import concourse.bass as bass
import concourse.bacc as bacc
import concourse.mybir as mybir
import concourse.tile as tile
from concourse.bass_utils import run_bass_kernel_spmd


def _install_ntff_hook_shim():
    """antenv in this image lacks axon_hooks; bass_utils imports it whenever
    tracing is requested (including via BASS_TRACE env). Provide the
    documented ctypes-based hook so that path works instead of crashing."""
    try:
        import antenv.axon_hooks  # noqa: F401
        return
    except ImportError:
        pass
    import types, contextlib, ctypes, os

    so_path = "/opt/axon/libaxon_pjrt.so"
    hook = None
    if os.path.exists(so_path):
        try:
            lib = ctypes.CDLL(so_path)
            if hasattr(lib, "axon_start_nrt_profile"):
                lib.axon_start_nrt_profile.argtypes = [
                    ctypes.POINTER(ctypes.c_int64), ctypes.c_size_t]
                lib.axon_start_nrt_profile.restype = ctypes.c_int64
                lib.axon_stop_nrt_profile.argtypes = [ctypes.c_char_p]
                lib.axon_stop_nrt_profile.restype = ctypes.c_int64

                @contextlib.contextmanager
                def _hook(output_dir, device_ids):
                    import jax
                    jax.devices()
                    if device_ids:
                        ids = (ctypes.c_int64 * len(device_ids))(*device_ids)
                        rc = lib.axon_start_nrt_profile(ids, len(device_ids))
                    else:
                        rc = lib.axon_start_nrt_profile(None, 0)
                    if rc != 0:
                        raise RuntimeError(f"axon_start_nrt_profile rc={rc}")
                    try:
                        yield
                    finally:
                        n = lib.axon_stop_nrt_profile(str(output_dir).encode())
                        print(f"ntff profile: {n} file(s) in {output_dir}")

                hook = _hook
        except OSError:
            pass

    try:
        import antenv
    except ImportError:
        return
    m = types.ModuleType("antenv.axon_hooks")
    m.get_axon_ntff_profile_hook = (lambda h: (lambda: h))(hook)
    m.set_axon_ntff_profile_hook = lambda h: None
    sys.modules["antenv.axon_hooks"] = m
    antenv.axon_hooks = m


_install_ntff_hook_shim()

B, I, O, NUM, K = 512, 512, 512, 8, 3
NPLANES = 6          # t', t'^2, t'^3, f4, |t'^3|, f6
O_SPLIT, B_SPLIT = 4, 2
OQ = O // O_SPLIT    # 128 out dims per core
BH = B // B_SPLIT    # 256 batch rows per core
ICHUNKS = I // 128   # 4 partition chunks of the in_dim
FREE = ICHUNKS * BH  # 1024: feature-plane free dim (i-chunks stacked)
NCORES = O_SPLIT * B_SPLIT
XCHUNKS = 2          # x/feature free-dim pipeline chunks
CW = FREE // XCHUNKS

F32 = mybir.dt.float32
F16 = mybir.dt.float16
# w_d column-block order = matmul emission order (f4's plane last)
PLANE_ORDER = (0, 1, 2, 4, 5, 3)


def _basis_coeffs():
    """Exact expansion of basis_k (k=0..NUM+K-1) in {1,t',t'^2,t'^3,
    r8^3, r9^3, r10^3} where rj^3 = relu(t'-(j-9))^3.

    basis_k(x) = N(t - k) with N the cardinal cubic B-spline
    N(s) = sum_j (-1)^j C(4,j)/6 * relu(s-j)^3.  For t in [7,11) knots
    p <= 7 are always active (pure cubics -> poly part around t'=t-9),
    knots p in {8,9,10} stay as relu kinks; p >= 11 never activates.
    """
    from math import comb

    nb = NUM + K
    C = np.zeros((7, nb))
    for k in range(nb):
        for j in range(5):
            w = ((-1) ** j) * comb(4, j) / 6.0
            p = k + j
            if p >= 11:
                continue
            if p <= 7:
                c = 9.0 - p
                C[0, k] += w * c ** 3
                C[1, k] += w * 3 * c ** 2
                C[2, k] += w * 3 * c
                C[3, k] += w
            else:
                C[4 + (p - 8), k] += w
    return C


def _silu_cubic(a1, a0):
    """Least-squares cubic of silu on [0,1), expressed in powers of
    t' = a1*x + a0.  Returns q[0..3]."""
    xs = np.linspace(0.0, 1.0, 4001)
    silu = xs / (1.0 + np.exp(-xs))
    V = np.vander(xs, 4, increasing=True)
    cfit, *_ = np.linalg.lstsq(V, silu, rcond=None)
    lin = np.array([-a0 / a1, 1.0 / a1])     # x as a poly in t'
    q = np.zeros(4)
    xp = np.array([1.0])
    for ci in cfit:
        q[: len(xp)] += ci * xp
        xp = np.convolve(xp, lin)
    return q


def _fold_weights(grid, coef, scale_base, scale_sp, mask):
    g0 = float(grid[0, 0])
    h = float(grid[0, 1]) - g0
    a1 = 1.0 / h
    a0 = -g0 / h - 9.0
    C = _basis_coeffs()                                        # (7, 11)
    A = (mask.astype(np.float64) * scale_sp.astype(np.float64))[:, :, None] \
        * coef.astype(np.float64)                              # (I, O, 11)
    W = np.einsum("fk,iok->fio", C[1:7], A)    # [t',t'2,t'3,r8,r9,r10]
    bias = np.einsum("k,iok->o", C[0], A)
    # silu -> cubic in t', folded into the monomial planes
    q = _silu_cubic(a1, a0)
    SB = mask.astype(np.float64) * scale_base.astype(np.float64)
    W[0] += SB * q[1]
    W[1] += SB * q[2]
    W[2] += SB * q[3]
    bias = bias + SB.sum(axis=0) * q[0]
    return W, bias, a1, a0


def _build_nc(a1, a0):
    AF = mybir.ActivationFunctionType
    AO = mybir.AluOpType

    nc = bacc.Bacc("TRN2", target_bir_lowering=False, debug=False)
    xt_d = nc.dram_tensor("xt", [128, FREE], F16, kind="ExternalInput").ap()
    w_d = nc.dram_tensor("w", [128, NPLANES * I], F16, kind="ExternalInput").ap()
    b_d = nc.dram_tensor("bias", [128, 1], F32, kind="ExternalInput").ap()
    o_d = nc.dram_tensor("out", [128, BH], F16, kind="ExternalOutput").ap()

    with tile.TileContext(nc) as tc:
        with (
            tc.tile_pool(name="main", bufs=1) as pool,
            tc.tile_pool(name="ps", bufs=1, space=bass.MemorySpace.PSUM) as pp,
        ):
            # PE power-throttle warmup: the PE ramps from K=4/8 to K=8/8
            # only after ~4us of sustained activity (see tensor-engine
            # guide, HAM).  Burn dummy accumulations into a scratch PSUM
            # bank while the input DMAs are in flight so the real matmuls
            # run at full rate.  The dummy memset is emitted first so the
            # warmup starts as early as possible.
            dummy = pool.tile([128, BH], F16, tag="dummy", name="dummy")
            nc.gpsimd.memset(dummy[:], 0.0)
            warm = pp.tile([128, BH], F32, tag="warm")
            NWARM = 20
            for i in range(NWARM):
                nc.tensor.matmul(
                    warm[:], dummy[:, :128], dummy[:],
                    start=(i == 0), stop=(i == NWARM - 1),
                )

            # x halves lead both HWDGE rings (SP + ACT) so both land ASAP;
            # weights follow as per-plane blocks, alternating rings in
            # matmul-need order [0,1,2,4,5,3].
            xs = pool.tile([128, FREE], F16, tag="xs")
            w_sb = pool.tile([128, NPLANES * I], F16, tag="w")
            bias_sb = pool.tile([128, 1], F32, tag="bias")
            # x halves lead both rings; w (laid out in matmul-need order
            # on the host) follows as one half per ring.  All DIRECT2D
            # issue must clear the ACT sequencer before x lands (~3.5us
            # in) or it delays the ACT Squares.
            nc.sync.dma_start(xs[:, :CW], xt_d[:, :CW])
            nc.scalar.dma_start(xs[:, CW:], xt_d[:, CW:])
            # first block small so the leading matmuls start right after
            # the PE warmup; rings stay byte-balanced (512KB each)
            nc.sync.dma_start(w_sb[:, :2 * I], w_d[:, :2 * I])
            nc.sync.dma_start(w_sb[:, 2 * I:3 * I], w_d[:, 2 * I:3 * I])
            nc.scalar.dma_start(w_sb[:, 3 * I:], w_d[:, 3 * I:])
            nc.scalar.dma_start(bias_sb[:], b_d[:])

            b2c = pool.tile([128, 1], F32, tag="b2c", name="b2c")
            b8c = pool.tile([128, 1], F32, tag="b8c", name="b8c")
            b10c = pool.tile([128, 1], F32, tag="b10c", name="b10c")
            nc.vector.memset(b2c[:], a0)
            nc.vector.memset(b8c[:], a0 + 1.0)
            nc.vector.memset(b10c[:], a0 - 1.0)

            planes = [
                pool.tile([128, FREE], F16, tag=f"pl{j}", name=f"pl{j}")
                for j in range(NPLANES)
            ]
            tp, p2, p3, f4, f5, f6 = planes
            a8 = pool.tile([128, FREE], F16, tag="a8")
            a10 = pool.tile([128, FREE], F16, tag="a10")
            s8 = pool.tile([128, FREE], F16, tag="s8")
            s10 = pool.tile([128, FREE], F16, tag="s10")

            acc = pp.tile([128, BH], F32, tag="acc")
            n = 0
            for k in range(XCHUNKS):
                sl = slice(k * CW, (k + 1) * CW)
                # ACT: the three biased squares (p2 first: it feeds p3)
                nc.scalar.activation(p2[:, sl], xs[:, sl], AF.Square,
                                     bias=b2c[:], scale=a1)
                nc.scalar.activation(s10[:, sl], xs[:, sl], AF.Square,
                                     bias=b10c[:], scale=a1)
                nc.scalar.activation(s8[:, sl], xs[:, sl], AF.Square,
                                     bias=b8c[:], scale=a1)
                # DVE: t', relu shifts, power chain; relu(t')^3 = relu(t'^3)
                nc.vector.tensor_scalar(tp[:, sl], xs[:, sl], a1, a0,
                                        AO.mult, AO.add)
                nc.vector.tensor_scalar(a10[:, sl], tp[:, sl], -1.0, 0.0,
                                        AO.add, AO.max)
                nc.vector.tensor_scalar(a8[:, sl], tp[:, sl], 1.0, 0.0,
                                        AO.add, AO.max)
                nc.vector.tensor_mul(p3[:, sl], p2[:, sl], tp[:, sl])
                nc.vector.tensor_scalar(f5[:, sl], p3[:, sl], 0.0, None,
                                        AO.max)
                nc.vector.tensor_mul(f6[:, sl], s10[:, sl], a10[:, sl])
                nc.vector.tensor_mul(f4[:, sl], s8[:, sl], a8[:, sl])
                # matmuls for the i-chunks covered by this feature chunk,
                # emitted in plane-readiness order (f4 needs ACT's 3rd op);
                # w_sb blocks are laid out in this same order on the host
                for rank, f in enumerate(PLANE_ORDER):
                    for ic in range(2 * k, 2 * k + 2):
                        c = rank * ICHUNKS + ic
                        nc.tensor.matmul(
                            acc[:],
                            w_sb[:, c * 128:(c + 1) * 128],
                            planes[f][:, ic * BH:(ic + 1) * BH],
                            start=(n == 0),
                            stop=(n == NPLANES * ICHUNKS - 1),
                        )
                        n += 1

            outs = pool.tile([128, BH], F16, tag="outs")
            nc.scalar.activation(outs[:], acc[:], AF.Identity,
                                 bias=bias_sb[:, 0:1])
            nc.sync.dma_start(o_d[:], outs[:])

    nc.compile()
    return nc


def _make_in_maps(x, W_all, bias):
    """Slice + layout-swizzle the folded weights and x for the 8 cores."""
    in_maps = []
    for c in range(NCORES):
        oq, bh = c // B_SPLIT, c % B_SPLIT
        xs = x[bh * BH:(bh + 1) * BH, :]                       # (BH, I)
        xt = np.ascontiguousarray(
            xs.T.reshape(ICHUNKS, 128, BH).transpose(1, 0, 2).reshape(128, FREE)
        ).astype(np.float16)
        Wq = W_all[PLANE_ORDER, :, oq * OQ:(oq + 1) * OQ]      # (6, I, OQ)
        w = np.ascontiguousarray(
            Wq.reshape(NPLANES, ICHUNKS, 128, OQ)
            .transpose(2, 0, 1, 3)
            .reshape(128, NPLANES * I)
        ).astype(np.float16)
        b = np.ascontiguousarray(
            bias[oq * OQ:(oq + 1) * OQ, None]
        ).astype(np.float32)
        in_maps.append({"xt": xt, "w": w, "bias": b})
    return in_maps


def _assemble(results):
    full = np.empty((B, O), np.float32)
    for c in range(NCORES):
        oq, bh = c // B_SPLIT, c % B_SPLIT
        full[bh * BH:(bh + 1) * BH, oq * OQ:(oq + 1) * OQ] = \
            results[c]["out"].T.astype(np.float32)
    return full


_CACHED = {}


def _get_nc(a1, a0):
    key = (a1, a0)
    if key not in _CACHED:
        _CACHED[key] = _build_nc(a1, a0)
    return _CACHED[key]


def kernel(x, grid, coef, scale_base, scale_sp, mask, _run_kwargs=None):
    x = np.asarray(x)
    W_all, bias, a1, a0 = _fold_weights(
        np.asarray(grid), np.asarray(coef), np.asarray(scale_base),
        np.asarray(scale_sp), np.asarray(mask)
    )
    nc = _get_nc(a1, a0)
    in_maps = _make_in_maps(x, W_all, bias)
    res = run_bass_kernel_spmd(
        nc, in_maps, core_ids=list(range(NCORES)), **(_run_kwargs or {})
    )
    out = _assemble(res.results)
    if _run_kwargs:
        kernel.last_result = res
    return out


# revision 31
# speedup vs baseline: 1.0870x; 1.0870x over previous
"""Trainium2 Bass kernel for the KAN layer (nn_KANLayer).

Math restructure
----------------
Reference computes, for x in [0,1) on a uniform extended B-spline grid
(g0 = grid[0,0], h = grid spacing, t = (x-g0)/h in [7,11), t' = t-9):

  y[b,o] = sum_i mask[i,o]*(scale_base[i,o]*silu(x[b,i])
                            + scale_sp[i,o]*sum_k basis_k(x[b,i])*coef[i,o,k])

On the restricted domain every cubic B-spline basis function is an exact
linear combination of the monomials {1, t', t'^2, t'^3} plus the three
relu-kink cubics relu(t'+1)^3, relu(t')^3, relu(t'-1)^3.  silu(x) on
[0,1) is replaced by its least-squares cubic (max err ~2e-4, well under
the 2e-2 gate) and folded into the monomial planes.  Since cubing
preserves sign, relu(t')^3 = relu(t'^3) costs one max after t'^3, and
relu(u)^3 = u^2*relu(u) gives the shifted kinks from an ACT Square and a
relu.  Net: SIX feature planes

  phi = [t', t'^2, t'^3, relu(t'+1)^3, relu(t')^3, relu(t'-1)^3]

and the whole layer is one accumulated matmul  y = Phi(x) @ W_fold + bias
with all scale factors folded into W on the host.

Sharding: out_dim split x4, batch split x2 -> 8 cores, no collectives.
Per core: x [128,1024] fp16 (2 chunked DMAs, SP ring) + W [128,3072] fp16
(2 DMAs, ACT ring).  Features are computed in 2 free-dim chunks balanced
across DVE (t', power chain, abs shifts), ACT (biased Squares) and Pool
(the two independent tensor_tensor products), so the 24 accumulated
matmuls (PE, fp16) start while the second chunk is still streaming.
Output is fp16 (cast to fp32 on host), bias added on ACT from PSUM.

Host does only weight folding, slicing, dtype casts and layout swizzles;
all per-token math (features, matmul, bias) runs on device.
"""

import sys

for _p in ("/opt/trn_rl_repo", "/opt/trn_rl_repo/concourse"):
    if _p not in sys.path:
        sys.path.insert(0, _p)

import numpy as np

import concourse.bass as bass
import concourse.bacc as bacc
import concourse.mybir as mybir
import concourse.tile as tile
from concourse.bass_utils import run_bass_kernel_spmd


def _install_ntff_hook_shim():
    """antenv in this image lacks axon_hooks; bass_utils imports it whenever
    tracing is requested (including via BASS_TRACE env). Provide the
    documented ctypes-based hook so that path works instead of crashing."""
    try:
        import antenv.axon_hooks  # noqa: F401
        return
    except ImportError:
        pass
    import types, contextlib, ctypes, os

    so_path = "/opt/axon/libaxon_pjrt.so"
    hook = None
    if os.path.exists(so_path):
        try:
            lib = ctypes.CDLL(so_path)
            if hasattr(lib, "axon_start_nrt_profile"):
                lib.axon_start_nrt_profile.argtypes = [
                    ctypes.POINTER(ctypes.c_int64), ctypes.c_size_t]
                lib.axon_start_nrt_profile.restype = ctypes.c_int64
                lib.axon_stop_nrt_profile.argtypes = [ctypes.c_char_p]
                lib.axon_stop_nrt_profile.restype = ctypes.c_int64

                @contextlib.contextmanager
                def _hook(output_dir, device_ids):
                    import jax
                    jax.devices()
                    if device_ids:
                        ids = (ctypes.c_int64 * len(device_ids))(*device_ids)
                        rc = lib.axon_start_nrt_profile(ids, len(device_ids))
                    else:
                        rc = lib.axon_start_nrt_profile(None, 0)
                    if rc != 0:
                        raise RuntimeError(f"axon_start_nrt_profile rc={rc}")
                    try:
                        yield
                    finally:
                        n = lib.axon_stop_nrt_profile(str(output_dir).encode())
                        print(f"ntff profile: {n} file(s) in {output_dir}")

                hook = _hook
        except OSError:
            pass

    try:
        import antenv
    except ImportError:
        return
    m = types.ModuleType("antenv.axon_hooks")
    m.get_axon_ntff_profile_hook = (lambda h: (lambda: h))(hook)
    m.set_axon_ntff_profile_hook = lambda h: None
    sys.modules["antenv.axon_hooks"] = m
    antenv.axon_hooks = m


_install_ntff_hook_shim()

B, I, O, NUM, K = 512, 512, 512, 8, 3
NPLANES = 6          # t', t'^2, t'^3, f4, |t'^3|, f6
O_SPLIT, B_SPLIT = 4, 2
OQ = O // O_SPLIT    # 128 out dims per core
BH = B // B_SPLIT    # 256 batch rows per core
ICHUNKS = I // 128   # 4 partition chunks of the in_dim
FREE = ICHUNKS * BH  # 1024: feature-plane free dim (i-chunks stacked)
NCORES = O_SPLIT * B_SPLIT
XCHUNKS = 2          # x/feature free-dim pipeline chunks
CW = FREE // XCHUNKS

F32 = mybir.dt.float32
F16 = mybir.dt.float16
# w_d column-block order = matmul emission order (f4's plane last)
PLANE_ORDER = (0, 1, 2, 4, 5, 3)


def _basis_coeffs():
    """Exact expansion of basis_k (k=0..NUM+K-1) in {1,t',t'^2,t'^3,
    r8^3, r9^3, r10^3} where rj^3 = relu(t'-(j-9))^3.

    basis_k(x) = N(t - k) with N the cardinal cubic B-spline
    N(s) = sum_j (-1)^j C(4,j)/6 * relu(s-j)^3.  For t in [7,11) knots
    p <= 7 are always active (pure cubics -> poly part around t'=t-9),
    knots p in {8,9,10} stay as relu kinks; p >= 11 never activates.
    """
    from math import comb

    nb = NUM + K
    C = np.zeros((7, nb))
    for k in range(nb):
        for j in range(5):
            w = ((-1) ** j) * comb(4, j) / 6.0
            p = k + j
            if p >= 11:
                continue
            if p <= 7:
                c = 9.0 - p
                C[0, k] += w * c ** 3
                C[1, k] += w * 3 * c ** 2
                C[2, k] += w * 3 * c
                C[3, k] += w
            else:
                C[4 + (p - 8), k] += w
    return C


def _silu_cubic(a1, a0):
    """Least-squares cubic of silu on [0,1), expressed in powers of
    t' = a1*x + a0.  Returns q[0..3]."""
    xs = np.linspace(0.0, 1.0, 4001)
    silu = xs / (1.0 + np.exp(-xs))
    V = np.vander(xs, 4, increasing=True)
    cfit, *_ = np.linalg.lstsq(V, silu, rcond=None)
    lin = np.array([-a0 / a1, 1.0 / a1])     # x as a poly in t'
    q = np.zeros(4)
    xp = np.array([1.0])
    for ci in cfit:
        q[: len(xp)] += ci * xp
        xp = np.convolve(xp, lin)
    return q


def _fold_weights(grid, coef, scale_base, scale_sp, mask):
    g0 = float(grid[0, 0])
    h = float(grid[0, 1]) - g0
    a1 = 1.0 / h
    a0 = -g0 / h - 9.0
    C = _basis_coeffs()                                        # (7, 11)
    A = (mask.astype(np.float64) * scale_sp.astype(np.float64))[:, :, None] \
        * coef.astype(np.float64)                              # (I, O, 11)
    W = np.einsum("fk,iok->fio", C[1:7], A)    # [t',t'2,t'3,r8,r9,r10]
    bias = np.einsum("k,iok->o", C[0], A)
    # silu -> cubic in t', folded into the monomial planes
    q = _silu_cubic(a1, a0)
    SB = mask.astype(np.float64) * scale_base.astype(np.float64)
    W[0] += SB * q[1]
    W[1] += SB * q[2]
    W[2] += SB * q[3]
    bias = bias + SB.sum(axis=0) * q[0]
    return W, bias, a1, a0


def _build_nc(a1, a0):
    AF = mybir.ActivationFunctionType
    AO = mybir.AluOpType

    nc = bacc.Bacc("TRN2", target_bir_lowering=False, debug=False)
    xt_d = nc.dram_tensor("xt", [128, FREE], F16, kind="ExternalInput").ap()
    # w carries the bias as one extra fp16 column in its last block
    w_d = nc.dram_tensor("w", [128, NPLANES * I + 1], F16,
                         kind="ExternalInput").ap()
    o_d = nc.dram_tensor("out", [128, BH], F16, kind="ExternalOutput").ap()

    with tile.TileContext(nc) as tc:
        with (
            tc.tile_pool(name="main", bufs=1) as pool,
            tc.tile_pool(name="ps", bufs=1, space=bass.MemorySpace.PSUM) as pp,
        ):
            # PE power-throttle warmup: the PE ramps from K=4/8 to K=8/8
            # only after ~4us of sustained activity (see tensor-engine
            # guide, HAM).  Burn dummy accumulations into a scratch PSUM
            # bank while the input DMAs are in flight so the real matmuls
            # run at full rate.  The dummy memset is emitted first so the
            # warmup starts as early as possible.
            dummy = pool.tile([128, BH], F16, tag="dummy", name="dummy")
            nc.gpsimd.memset(dummy[:], 0.0)
            warm = pp.tile([128, BH], F32, tag="warm")
            NWARM = 20
            for i in range(NWARM):
                nc.tensor.matmul(
                    warm[:], dummy[:, :128], dummy[:],
                    start=(i == 0), stop=(i == NWARM - 1),
                )

            # x halves lead both HWDGE rings (SP + ACT) so both land ASAP.
            # Weights (host-laid-out in matmul-need order [0,1,2,4,5,3])
            # follow as one 128KB block per plane, alternating rings, so
            # landings pace the PE's in-order consumption; both rings
            # carry 4 issues / 512KB and clear before x lands.  The bias
            # rides as one extra fp16 column in the last block.
            xs = pool.tile([128, FREE], F16, tag="xs")
            w_sb = pool.tile([128, NPLANES * I + 1], F16, tag="w")
            nc.sync.dma_start(xs[:, :CW], xt_d[:, :CW])
            nc.scalar.dma_start(xs[:, CW:], xt_d[:, CW:])
            for rank in range(NPLANES):
                lo = rank * I
                hi = (rank + 1) * I + (1 if rank == NPLANES - 1 else 0)
                eng = nc.sync if rank % 2 == 0 else nc.scalar
                eng.dma_start(w_sb[:, lo:hi], w_d[:, lo:hi])
            bias_sb = w_sb[:, NPLANES * I:NPLANES * I + 1]

            b2c = pool.tile([128, 1], F32, tag="b2c", name="b2c")
            b8c = pool.tile([128, 1], F32, tag="b8c", name="b8c")
            b10c = pool.tile([128, 1], F32, tag="b10c", name="b10c")
            nc.vector.memset(b2c[:], a0)
            nc.vector.memset(b8c[:], a0 + 1.0)
            nc.vector.memset(b10c[:], a0 - 1.0)

            planes = [
                pool.tile([128, FREE], F16, tag=f"pl{j}", name=f"pl{j}")
                for j in range(NPLANES)
            ]
            tp, p2, p3, f4, f5, f6 = planes
            a8 = pool.tile([128, FREE], F16, tag="a8")
            a10 = pool.tile([128, FREE], F16, tag="a10")
            s8 = pool.tile([128, FREE], F16, tag="s8")
            s10 = pool.tile([128, FREE], F16, tag="s10")

            acc = pp.tile([128, BH], F32, tag="acc")
            n = 0
            for k in range(XCHUNKS):
                sl = slice(k * CW, (k + 1) * CW)
                # ACT: the three biased squares (p2 first: it feeds p3)
                nc.scalar.activation(p2[:, sl], xs[:, sl], AF.Square,
                                     bias=b2c[:], scale=a1)
                nc.scalar.activation(s10[:, sl], xs[:, sl], AF.Square,
                                     bias=b10c[:], scale=a1)
                nc.scalar.activation(s8[:, sl], xs[:, sl], AF.Square,
                                     bias=b8c[:], scale=a1)
                # DVE: t', relu shifts, power chain; relu(t')^3 = relu(t'^3)
                nc.vector.tensor_scalar(tp[:, sl], xs[:, sl], a1, a0,
                                        AO.mult, AO.add)
                nc.vector.tensor_scalar(a10[:, sl], tp[:, sl], -1.0, 0.0,
                                        AO.add, AO.max)
                nc.vector.tensor_scalar(a8[:, sl], tp[:, sl], 1.0, 0.0,
                                        AO.add, AO.max)
                nc.vector.tensor_mul(p3[:, sl], p2[:, sl], tp[:, sl])
                nc.vector.tensor_scalar(f5[:, sl], p3[:, sl], 0.0, None,
                                        AO.max)
                nc.vector.tensor_mul(f6[:, sl], s10[:, sl], a10[:, sl])
                nc.vector.tensor_mul(f4[:, sl], s8[:, sl], a8[:, sl])
                # matmuls for the i-chunks covered by this feature chunk,
                # emitted in plane-readiness order (f4 needs ACT's 3rd op);
                # w_sb blocks are laid out in this same order on the host
                for rank, f in enumerate(PLANE_ORDER):
                    for ic in range(2 * k, 2 * k + 2):
                        c = rank * ICHUNKS + ic
                        nc.tensor.matmul(
                            acc[:],
                            w_sb[:, c * 128:(c + 1) * 128],
                            planes[f][:, ic * BH:(ic + 1) * BH],
                            start=(n == 0),
                            stop=(n == NPLANES * ICHUNKS - 1),
                        )
                        n += 1

            outs = pool.tile([128, BH], F16, tag="outs")
            nc.scalar.activation(outs[:], acc[:], AF.Identity,
                                 bias=bias_sb)
            nc.sync.dma_start(o_d[:], outs[:])

    nc.compile()
    return nc


def _make_in_maps(x, W_all, bias):
    """Slice + layout-swizzle the folded weights and x for the 8 cores."""
    in_maps = []
    for c in range(NCORES):
        oq, bh = c // B_SPLIT, c % B_SPLIT
        xs = x[bh * BH:(bh + 1) * BH, :]                       # (BH, I)
        xt = np.ascontiguousarray(
            xs.T.reshape(ICHUNKS, 128, BH).transpose(1, 0, 2).reshape(128, FREE)
        ).astype(np.float16)
        Wq = W_all[PLANE_ORDER, :, oq * OQ:(oq + 1) * OQ]      # (6, I, OQ)
        w = np.empty((128, NPLANES * I + 1), np.float16)
        w[:, :NPLANES * I] = (
            Wq.reshape(NPLANES, ICHUNKS, 128, OQ)
            .transpose(2, 0, 1, 3)
            .reshape(128, NPLANES * I)
        ).astype(np.float16)
        w[:, NPLANES * I] = bias[oq * OQ:(oq + 1) * OQ].astype(np.float16)
        in_maps.append({"xt": xt, "w": w})
    return in_maps


def _assemble(results):
    full = np.empty((B, O), np.float32)
    for c in range(NCORES):
        oq, bh = c // B_SPLIT, c % B_SPLIT
        full[bh * BH:(bh + 1) * BH, oq * OQ:(oq + 1) * OQ] = \
            results[c]["out"].T.astype(np.float32)
    return full


_CACHED = {}


def _get_nc(a1, a0):
    key = (a1, a0)
    if key not in _CACHED:
        _CACHED[key] = _build_nc(a1, a0)
    return _CACHED[key]


def kernel(x, grid, coef, scale_base, scale_sp, mask, _run_kwargs=None):
    x = np.asarray(x)
    W_all, bias, a1, a0 = _fold_weights(
        np.asarray(grid), np.asarray(coef), np.asarray(scale_base),
        np.asarray(scale_sp), np.asarray(mask)
    )
    nc = _get_nc(a1, a0)
    in_maps = _make_in_maps(x, W_all, bias)
    res = run_bass_kernel_spmd(
        nc, in_maps, core_ids=list(range(NCORES)), **(_run_kwargs or {})
    )
    out = _assemble(res.results)
    if _run_kwargs:
        kernel.last_result = res
    return out


# revision 32
# speedup vs baseline: 1.2290x; 1.1307x over previous
"""Trainium2 Bass kernel for the KAN layer (nn_KANLayer).

Math restructure
----------------
Reference computes, for x in [0,1) on a uniform extended B-spline grid
(g0 = grid[0,0], h = grid spacing, t = (x-g0)/h in [7,11), t' = t-9):

  y[b,o] = sum_i mask[i,o]*(scale_base[i,o]*silu(x[b,i])
                            + scale_sp[i,o]*sum_k basis_k(x[b,i])*coef[i,o,k])

On the restricted domain every cubic B-spline basis function is an exact
linear combination of the monomials {1, t', t'^2, t'^3} plus the three
relu-kink cubics relu(t'+1)^3, relu(t')^3, relu(t'-1)^3.  silu(x) on
[0,1) is replaced by its least-squares cubic (max err ~2e-4, well under
the 2e-2 gate) and folded into the monomial planes.  Since cubing
preserves sign, relu(t')^3 = relu(t'^3) costs one max after t'^3, and
relu(u)^3 = u^2*relu(u) gives the shifted kinks from an ACT Square and a
relu.  Net: SIX feature planes

  phi = [t', t'^2, t'^3, relu(t'+1)^3, relu(t')^3, relu(t'-1)^3]

and the whole layer is one accumulated matmul  y = Phi(x) @ W_fold + bias
with all scale factors folded into W on the host.

Sharding: out_dim split x4, batch split x2 -> 8 cores, no collectives.
Per core and schedule (all fp16 on the wire, fp32 PSUM):
 - The PE power-throttle starts at half rate (K=4/8) and only reaches
   full rate after ~4us of sustained activity, so a chain of dummy
   warmup matmuls into a scratch PSUM bank runs during the DMA wait;
   the 24 real matmuls then run at ~110ns instead of ~213ns each.
 - x halves lead both HWDGE rings (SP + ACT) and the weights follow as
   one 128KB block per plane, alternating rings in matmul-need order,
   so weight landings pace the PE's strictly in-order consumption.
   The bias rides as one extra fp16 column of the weight tensor.  The
   ACT ring issues only 4 DMAs so its sequencer is free before x lands.
 - Features run in 2 free-dim chunks split across DVE (t', relu shifts,
   power chain, relu(t'^3)) and ACT (the three biased Squares); the
   GpSimd/Pool engine is avoided entirely - it shares an SBUF port with
   DVE and stalls it ~3x.
 - Output is fp16 (cast to fp32 on host), bias added via one ACT
   Identity from PSUM.

Host does only weight folding, slicing, dtype casts and layout swizzles;
all per-token math (features, matmul, bias) runs on device.
"""

import sys

for _p in ("/opt/trn_rl_repo", "/opt/trn_rl_repo/concourse"):
    if _p not in sys.path:
        sys.path.insert(0, _p)

import numpy as np

import concourse.bass as bass
import concourse.bacc as bacc
import concourse.mybir as mybir
import concourse.tile as tile
from concourse.bass_utils import run_bass_kernel_spmd


def _install_ntff_hook_shim():
    """antenv in this image lacks axon_hooks; bass_utils imports it whenever
    tracing is requested (including via BASS_TRACE env). Provide the
    documented ctypes-based hook so that path works instead of crashing."""
    try:
        import antenv.axon_hooks  # noqa: F401
        return
    except ImportError:
        pass
    import types, contextlib, ctypes, os

    so_path = "/opt/axon/libaxon_pjrt.so"
    hook = None
    if os.path.exists(so_path):
        try:
            lib = ctypes.CDLL(so_path)
            if hasattr(lib, "axon_start_nrt_profile"):
                lib.axon_start_nrt_profile.argtypes = [
                    ctypes.POINTER(ctypes.c_int64), ctypes.c_size_t]
                lib.axon_start_nrt_profile.restype = ctypes.c_int64
                lib.axon_stop_nrt_profile.argtypes = [ctypes.c_char_p]
                lib.axon_stop_nrt_profile.restype = ctypes.c_int64

                @contextlib.contextmanager
                def _hook(output_dir, device_ids):
                    import jax
                    jax.devices()
                    if device_ids:
                        ids = (ctypes.c_int64 * len(device_ids))(*device_ids)
                        rc = lib.axon_start_nrt_profile(ids, len(device_ids))
                    else:
                        rc = lib.axon_start_nrt_profile(None, 0)
                    if rc != 0:
                        raise RuntimeError(f"axon_start_nrt_profile rc={rc}")
                    try:
                        yield
                    finally:
                        n = lib.axon_stop_nrt_profile(str(output_dir).encode())
                        print(f"ntff profile: {n} file(s) in {output_dir}")

                hook = _hook
        except OSError:
            pass

    try:
        import antenv
    except ImportError:
        return
    m = types.ModuleType("antenv.axon_hooks")
    m.get_axon_ntff_profile_hook = (lambda h: (lambda: h))(hook)
    m.set_axon_ntff_profile_hook = lambda h: None
    sys.modules["antenv.axon_hooks"] = m
    antenv.axon_hooks = m


_install_ntff_hook_shim()

B, I, O, NUM, K = 512, 512, 512, 8, 3
NPLANES = 6          # t', t'^2, t'^3, f4, |t'^3|, f6
O_SPLIT, B_SPLIT = 4, 2
OQ = O // O_SPLIT    # 128 out dims per core
BH = B // B_SPLIT    # 256 batch rows per core
ICHUNKS = I // 128   # 4 partition chunks of the in_dim
FREE = ICHUNKS * BH  # 1024: feature-plane free dim (i-chunks stacked)
NCORES = O_SPLIT * B_SPLIT
XCHUNKS = 2          # x/feature free-dim pipeline chunks
CW = FREE // XCHUNKS

F32 = mybir.dt.float32
F16 = mybir.dt.float16
# w_d column-block order = matmul emission order (f4's plane last)
PLANE_ORDER = (0, 1, 2, 4, 5, 3)


def _basis_coeffs():
    """Exact expansion of basis_k (k=0..NUM+K-1) in {1,t',t'^2,t'^3,
    r8^3, r9^3, r10^3} where rj^3 = relu(t'-(j-9))^3.

    basis_k(x) = N(t - k) with N the cardinal cubic B-spline
    N(s) = sum_j (-1)^j C(4,j)/6 * relu(s-j)^3.  For t in [7,11) knots
    p <= 7 are always active (pure cubics -> poly part around t'=t-9),
    knots p in {8,9,10} stay as relu kinks; p >= 11 never activates.
    """
    from math import comb

    nb = NUM + K
    C = np.zeros((7, nb))
    for k in range(nb):
        for j in range(5):
            w = ((-1) ** j) * comb(4, j) / 6.0
            p = k + j
            if p >= 11:
                continue
            if p <= 7:
                c = 9.0 - p
                C[0, k] += w * c ** 3
                C[1, k] += w * 3 * c ** 2
                C[2, k] += w * 3 * c
                C[3, k] += w
            else:
                C[4 + (p - 8), k] += w
    return C


def _silu_cubic(a1, a0):
    """Least-squares cubic of silu on [0,1), expressed in powers of
    t' = a1*x + a0.  Returns q[0..3]."""
    xs = np.linspace(0.0, 1.0, 4001)
    silu = xs / (1.0 + np.exp(-xs))
    V = np.vander(xs, 4, increasing=True)
    cfit, *_ = np.linalg.lstsq(V, silu, rcond=None)
    lin = np.array([-a0 / a1, 1.0 / a1])     # x as a poly in t'
    q = np.zeros(4)
    xp = np.array([1.0])
    for ci in cfit:
        q[: len(xp)] += ci * xp
        xp = np.convolve(xp, lin)
    return q


def _fold_weights(grid, coef, scale_base, scale_sp, mask):
    g0 = float(grid[0, 0])
    h = float(grid[0, 1]) - g0
    a1 = 1.0 / h
    a0 = -g0 / h - 9.0
    C = _basis_coeffs()                                        # (7, 11)
    A = (mask.astype(np.float64) * scale_sp.astype(np.float64))[:, :, None] \
        * coef.astype(np.float64)                              # (I, O, 11)
    W = np.einsum("fk,iok->fio", C[1:7], A)    # [t',t'2,t'3,r8,r9,r10]
    bias = np.einsum("k,iok->o", C[0], A)
    # silu -> cubic in t', folded into the monomial planes
    q = _silu_cubic(a1, a0)
    SB = mask.astype(np.float64) * scale_base.astype(np.float64)
    W[0] += SB * q[1]
    W[1] += SB * q[2]
    W[2] += SB * q[3]
    bias = bias + SB.sum(axis=0) * q[0]
    return W, bias, a1, a0


def _build_nc(a1, a0):
    AF = mybir.ActivationFunctionType
    AO = mybir.AluOpType

    nc = bacc.Bacc("TRN2", target_bir_lowering=False, debug=False)
    xt_d = nc.dram_tensor("xt", [128, FREE], F16, kind="ExternalInput").ap()
    # w carries the bias as one extra fp16 column in its last block
    w_d = nc.dram_tensor("w", [128, NPLANES * I + 1], F16,
                         kind="ExternalInput").ap()
    o_d = nc.dram_tensor("out", [128, BH], F16, kind="ExternalOutput").ap()

    with tile.TileContext(nc) as tc:
        with (
            tc.tile_pool(name="main", bufs=1) as pool,
            tc.tile_pool(name="ps", bufs=1, space=bass.MemorySpace.PSUM) as pp,
        ):
            # PE power-throttle warmup: the PE ramps from K=4/8 to K=8/8
            # only after ~4us of sustained activity (see tensor-engine
            # guide, HAM).  Burn dummy accumulations into a scratch PSUM
            # bank while the input DMAs are in flight so the real matmuls
            # run at full rate.  The dummy memset is emitted first so the
            # warmup starts as early as possible.
            dummy = pool.tile([128, BH], F16, tag="dummy", name="dummy")
            nc.gpsimd.memset(dummy[:], 0.0)
            warm = pp.tile([128, BH], F32, tag="warm")
            NWARM = 20
            for i in range(NWARM):
                nc.tensor.matmul(
                    warm[:], dummy[:, :128], dummy[:],
                    start=(i == 0), stop=(i == NWARM - 1),
                )

            # x halves lead both HWDGE rings (SP + ACT) so both land ASAP.
            # Weights (host-laid-out in matmul-need order [0,1,2,4,5,3])
            # follow as one 128KB block per plane, alternating rings, so
            # landings pace the PE's in-order consumption; both rings
            # carry 4 issues / 512KB and clear before x lands.  The bias
            # rides as one extra fp16 column in the last block.
            xs = pool.tile([128, FREE], F16, tag="xs")
            w_sb = pool.tile([128, NPLANES * I + 1], F16, tag="w")
            nc.sync.dma_start(xs[:, :CW], xt_d[:, :CW])
            nc.scalar.dma_start(xs[:, CW:], xt_d[:, CW:])
            for rank in range(NPLANES):
                lo = rank * I
                hi = (rank + 1) * I + (1 if rank == NPLANES - 1 else 0)
                eng = nc.sync if rank % 2 == 0 else nc.scalar
                eng.dma_start(w_sb[:, lo:hi], w_d[:, lo:hi])
            bias_sb = w_sb[:, NPLANES * I:NPLANES * I + 1]

            b2c = pool.tile([128, 1], F32, tag="b2c", name="b2c")
            b8c = pool.tile([128, 1], F32, tag="b8c", name="b8c")
            b10c = pool.tile([128, 1], F32, tag="b10c", name="b10c")
            nc.vector.memset(b2c[:], a0)
            nc.vector.memset(b8c[:], a0 + 1.0)
            nc.vector.memset(b10c[:], a0 - 1.0)

            planes = [
                pool.tile([128, FREE], F16, tag=f"pl{j}", name=f"pl{j}")
                for j in range(NPLANES)
            ]
            tp, p2, p3, f4, f5, f6 = planes
            a8 = pool.tile([128, FREE], F16, tag="a8")
            a10 = pool.tile([128, FREE], F16, tag="a10")
            s8 = pool.tile([128, FREE], F16, tag="s8")
            s10 = pool.tile([128, FREE], F16, tag="s10")

            acc = pp.tile([128, BH], F32, tag="acc")
            n = 0
            for k in range(XCHUNKS):
                sl = slice(k * CW, (k + 1) * CW)
                # ACT: the three biased squares (p2 first: it feeds p3)
                nc.scalar.activation(p2[:, sl], xs[:, sl], AF.Square,
                                     bias=b2c[:], scale=a1)
                nc.scalar.activation(s10[:, sl], xs[:, sl], AF.Square,
                                     bias=b10c[:], scale=a1)
                nc.scalar.activation(s8[:, sl], xs[:, sl], AF.Square,
                                     bias=b8c[:], scale=a1)
                # DVE: t', relu shifts, power chain; relu(t')^3 = relu(t'^3)
                nc.vector.tensor_scalar(tp[:, sl], xs[:, sl], a1, a0,
                                        AO.mult, AO.add)
                nc.vector.tensor_scalar(a10[:, sl], tp[:, sl], -1.0, 0.0,
                                        AO.add, AO.max)
                nc.vector.tensor_scalar(a8[:, sl], tp[:, sl], 1.0, 0.0,
                                        AO.add, AO.max)
                nc.vector.tensor_mul(p3[:, sl], p2[:, sl], tp[:, sl])
                nc.vector.tensor_scalar(f5[:, sl], p3[:, sl], 0.0, None,
                                        AO.max)
                nc.vector.tensor_mul(f6[:, sl], s10[:, sl], a10[:, sl])
                nc.vector.tensor_mul(f4[:, sl], s8[:, sl], a8[:, sl])
                # matmuls for the i-chunks covered by this feature chunk,
                # emitted in plane-readiness order (f4 needs ACT's 3rd op);
                # w_sb blocks are laid out in this same order on the host
                for rank, f in enumerate(PLANE_ORDER):
                    for ic in range(2 * k, 2 * k + 2):
                        c = rank * ICHUNKS + ic
                        nc.tensor.matmul(
                            acc[:],
                            w_sb[:, c * 128:(c + 1) * 128],
                            planes[f][:, ic * BH:(ic + 1) * BH],
                            start=(n == 0),
                            stop=(n == NPLANES * ICHUNKS - 1),
                        )
                        n += 1

            outs = pool.tile([128, BH], F16, tag="outs")
            nc.scalar.activation(outs[:], acc[:], AF.Identity,
                                 bias=bias_sb)
            nc.sync.dma_start(o_d[:], outs[:])

    nc.compile()
    return nc


def _make_in_maps(x, W_all, bias):
    """Slice + layout-swizzle the folded weights and x for the 8 cores."""
    in_maps = []
    for c in range(NCORES):
        oq, bh = c // B_SPLIT, c % B_SPLIT
        xs = x[bh * BH:(bh + 1) * BH, :]                       # (BH, I)
        xt = np.ascontiguousarray(
            xs.T.reshape(ICHUNKS, 128, BH).transpose(1, 0, 2).reshape(128, FREE)
        ).astype(np.float16)
        Wq = W_all[PLANE_ORDER, :, oq * OQ:(oq + 1) * OQ]      # (6, I, OQ)
        w = np.empty((128, NPLANES * I + 1), np.float16)
        w[:, :NPLANES * I] = (
            Wq.reshape(NPLANES, ICHUNKS, 128, OQ)
            .transpose(2, 0, 1, 3)
            .reshape(128, NPLANES * I)
        ).astype(np.float16)
        w[:, NPLANES * I] = bias[oq * OQ:(oq + 1) * OQ].astype(np.float16)
        in_maps.append({"xt": xt, "w": w})
    return in_maps


def _assemble(results):
    full = np.empty((B, O), np.float32)
    for c in range(NCORES):
        oq, bh = c // B_SPLIT, c % B_SPLIT
        full[bh * BH:(bh + 1) * BH, oq * OQ:(oq + 1) * OQ] = \
            results[c]["out"].T.astype(np.float32)
    return full


_CACHED = {}


def _get_nc(a1, a0):
    key = (a1, a0)
    if key not in _CACHED:
        _CACHED[key] = _build_nc(a1, a0)
    return _CACHED[key]


def kernel(x, grid, coef, scale_base, scale_sp, mask, _run_kwargs=None):
    x = np.asarray(x)
    W_all, bias, a1, a0 = _fold_weights(
        np.asarray(grid), np.asarray(coef), np.asarray(scale_base),
        np.asarray(scale_sp), np.asarray(mask)
    )
    nc = _get_nc(a1, a0)
    in_maps = _make_in_maps(x, W_all, bias)
    res = run_bass_kernel_spmd(
        nc, in_maps, core_ids=list(range(NCORES)), **(_run_kwargs or {})
    )
    out = _assemble(res.results)
    if _run_kwargs:
        kernel.last_result = res
    return out
